# revision 7
# baseline (speedup 1.0000x reference)
"""Trainium2 Bass kernel for a dense transformer block (pre-LN, causal attention, GELU FFN).

Sharding: 8 cores = 2 batches x 4 query-groups of 512 tokens. Every core
computes full K/V for its batch (communication-free); queries/proj/FFN are
token-parallel. All activations are kept feature-major ([d, tokens]) so no
on-device transposes are needed; LayerNorm is folded into host-prepared
weights plus on-device per-token column stats applied at PSUM evacuation.
"""

import sys

sys.path.insert(0, "/opt/trn_rl_repo")

import numpy as np
import ml_dtypes

import concourse.bass as bass
import concourse.tile as tile
from concourse import bacc, mybir
from concourse.bass import ts
from concourse.bass_utils import run_bass_kernel_spmd

F32 = mybir.dt.float32
F32R = mybir.dt.float32r
BF16 = mybir.dt.bfloat16
AF = mybir.ActivationFunctionType
ALU = mybir.AluOpType

EPS = 1e-5


class CFG:
    def __init__(self, D=1024, TB=2048, TQ=512, NH=16, HD=64, HFF=4096):
        self.D, self.TB, self.TQ, self.NH, self.HD, self.HFF = D, TB, TQ, NH, HD, HFF
        self.DT = D // 128          # d_model tiles
        self.FT = HFF // 128        # ffn tiles
        self.NTT = TB // 128        # key token tiles
        self.NBLK = TB // 512       # 512-token kv blocks
        self.VN = min(512, D)       # V matmul free width
        self.NVB = D // self.VN     # V col blocks
        self.HPV = self.VN // HD    # heads per V col block
        assert NH == 2 * self.DT and HD == 64




def build_nc(c: CFG):
    nc = bacc.Bacc()
    D, TB, TQ, DT, FT, NTT, NBLK = c.D, c.TB, c.TQ, c.DT, c.FT, c.NTT, c.NBLK

    xT = nc.dram_tensor("xT", [D, TB], F32R, kind="ExternalInput")
    xqT = nc.dram_tensor("xqT", [D, TQ], F32R, kind="ExternalInput")
    maskT = nc.dram_tensor("maskT", [TB, TQ], BF16, kind="ExternalInput")
    wq = nc.dram_tensor("wq", [D, D], F32R, kind="ExternalInput")
    wk = nc.dram_tensor("wk", [D, D], F32R, kind="ExternalInput")
    wv = nc.dram_tensor("wv", [D, D], F32R, kind="ExternalInput")
    pw = nc.dram_tensor("pw", [D, D], F32R, kind="ExternalInput")
    w1 = nc.dram_tensor("w1", [D, c.HFF], F32R, kind="ExternalInput")
    w2 = nc.dram_tensor("w2", [c.HFF, D], F32R, kind="ExternalInput")
    bq = nc.dram_tensor("bq", [128, DT], F32, kind="ExternalInput")
    bk = nc.dram_tensor("bk", [128, DT], F32, kind="ExternalInput")
    bv = nc.dram_tensor("bv", [1, D], F32, kind="ExternalInput")
    pb = nc.dram_tensor("pb", [128, DT], F32, kind="ExternalInput")
    b1 = nc.dram_tensor("b1", [128, FT], F32, kind="ExternalInput")
    b2 = nc.dram_tensor("b2", [128, DT], F32, kind="ExternalInput")
    outT = nc.dram_tensor("outT", [D, TQ], F32, kind="ExternalOutput")

    def dram3(t):  # [ (a p), m ] -> [p, a, m]
        return t.ap().rearrange("(a p) m -> p a m", p=128)

    with tile.TileContext(nc) as tc:
        with tc.tile_pool(name="persist", bufs=1) as P:
            aoT = P.tile([128, DT, TQ], F32R)
            x2T = P.tile([128, DT, TQ], F32R)
            bvb = P.tile([128, D], F32)
            mu_row = P.tile([1, TB], F32)
            rt = P.tile([128, NTT], F32)
            ones = P.tile([128, 1], F32R)
            eps_t = P.tile([1, 1], F32)
            id11 = P.tile([1, 1], F32)
            bq_t = P.tile([128, DT], F32)
            bk_t = P.tile([128, DT], F32)
            pb_t = P.tile([128, DT], F32)
            b2_t = P.tile([128, DT], F32)
            b1_t = P.tile([128, FT], F32)

            ones_f = P.tile([128, 1], F32)
            nc.vector.memset(ones_f[:], 1.0)
            nc.vector.tensor_copy(ones[:], ones_f[:])
            nc.vector.memset(eps_t[:], EPS)
            nc.vector.memset(id11[:], 1.0)
            nc.sync.dma_start(bq_t[:], bq.ap())
            nc.sync.dma_start(bk_t[:], bk.ap())
            nc.sync.dma_start(pb_t[:], pb.ap())
            nc.sync.dma_start(b2_t[:], b2.ap())
            nc.sync.dma_start(b1_t[:], b1.ap())
            bv_r = P.tile([1, D], F32)
            nc.sync.dma_start(bv_r[:], bv.ap())
            nc.gpsimd.partition_broadcast(bvb[:], bv_r[:])

            # ---- per-512-token-block LN stats + centering (feature-major) ----
            def stats_center(xb, W, mu_slice, r_slice, mu_bc, r_bc, sqp, stp):
                sum_ps = stp.tile([1, W], F32, tag="sum_ps")
                sq_ps = stp.tile([1, W], F32, tag="sq_ps")
                for k in range(DT):
                    nc.tensor.matmul(sum_ps[:], ones[:], xb[:, k, :],
                                     start=(k == 0), stop=(k == DT - 1))
                for k in range(DT):
                    sq = sqp.tile([128, W], F32R, tag="sq")
                    nc.scalar.square(sq[:], xb[:, k, :])
                    nc.tensor.matmul(sq_ps[:], ones[:], sq[:],
                                     start=(k == 0), stop=(k == DT - 1))
                nc.vector.tensor_scalar(mu_slice, sum_ps[:], 1.0 / D, None, ALU.mult)
                musq = sqp.tile([1, W], F32, tag="musq")
                nc.vector.tensor_tensor(musq[:], mu_slice, mu_slice, ALU.mult)
                var = sqp.tile([1, W], F32, tag="var")
                nc.vector.scalar_tensor_tensor(var[:], sq_ps[:], 1.0 / D, musq[:],
                                               ALU.mult, ALU.subtract)
                std = sqp.tile([1, W], F32, tag="std")
                nc.scalar.activation(std[:], var[:], AF.Sqrt, bias=eps_t[:])
                nc.vector.reciprocal(r_slice, std[:])
                nc.gpsimd.partition_broadcast(mu_bc[:], mu_slice)
                nc.gpsimd.partition_broadcast(r_bc[:], r_slice)
                for k in range(DT):
                    nc.vector.tensor_tensor(xb[:, k, :], xb[:, k, :], mu_bc[:],
                                            ALU.subtract)

            with tc.tile_pool(name="kvres", bufs=1) as KV:
                kT = KV.tile([128, DT, TB], BF16)
                V = KV.tile([128, NTT, c.NH * 65], BF16)
                qT = KV.tile([128, DT, TQ], BF16)
                vone = V[:].rearrange("p t (h c) -> p t h c", c=65)
                nc.vector.memset(vone[:, :, :, 64:65], 1.0)

                # ---------------- Phase Q ----------------
                with tc.tile_pool(name="phq", bufs=2) as PQ, \
                     tc.tile_pool(name="phq_ps", bufs=2, space="PSUM") as PQP:
                    xq = PQ.tile([128, DT, TQ], F32R, bufs=1)
                    nc.sync.dma_start(xq[:], dram3(xqT))
                    wq_t = PQ.tile([128, DT, D], F32R, bufs=1)
                    nc.sync.dma_start(wq_t[:], dram3(wq))
                    muq_r = PQ.tile([1, TQ], F32, bufs=1)
                    rq_r = PQ.tile([1, TQ], F32, bufs=1)
                    muq_bc = PQ.tile([128, TQ], F32, bufs=1)
                    rq_bc = PQ.tile([128, TQ], F32, bufs=1)
                    stats_center(xq, TQ, muq_r[:], rq_r[:], muq_bc, rq_bc, PQ, PQP)
                    for m in range(DT):
                        ps = PQP.tile([128, TQ], F32, tag="qps")
                        for k in range(DT):
                            nc.tensor.matmul(ps[:], wq_t[:, k, ts(m, 128)],
                                             xq[:, k, :],
                                             start=(k == 0), stop=(k == DT - 1))
                        ev = PQ.tile([128, TQ], F32, tag="qev", bufs=2)
                        nc.vector.tensor_tensor(ev[:], ps[:], rq_bc[:], ALU.mult)
                        nc.vector.tensor_scalar(qT[:, m, :], ev[:], bq_t[:, m:m + 1],
                                                None, ALU.add)

                # ---------------- Phase K ----------------
                with tc.tile_pool(name="phk", bufs=2) as PK, \
                     tc.tile_pool(name="phk_ps", bufs=2, space="PSUM") as PKP:
                    wk_t = PK.tile([128, DT, D], F32R, bufs=1)
                    nc.sync.dma_start(wk_t[:], dram3(wk))
                    r_row = PK.tile([1, TB], F32, bufs=1)
                    for blk in range(NBLK):
                        off = blk * 512
                        xb = PK.tile([128, DT, 512], F32R, tag="xb", bufs=1)
                        nc.sync.dma_start(xb[:], dram3(xT)[:, :, off:off + 512])
                        mu_bc = PK.tile([128, 512], F32, tag="mu_bc", bufs=2)
                        r_bc = PK.tile([128, 512], F32, tag="r_bc", bufs=2)
                        stats_center(xb, 512, mu_row[0:1, off:off + 512],
                                     r_row[0:1, off:off + 512], mu_bc, r_bc, PK, PKP)
                        # r token-major for the V pass
                        for tt in range(4):
                            g = blk * 4 + tt
                            rt_ps = PKP.tile([128, 1], F32, tag="rt_ps")
                            nc.tensor.transpose(
                                rt_ps[:], r_row[0:1, off + tt * 128:off + (tt + 1) * 128],
                                id11[:])
                            nc.vector.tensor_copy(rt[:, g:g + 1], rt_ps[:])
                        for m in range(DT):
                            ps = PKP.tile([128, 512], F32, tag="kps")
                            for k in range(DT):
                                nc.tensor.matmul(ps[:], wk_t[:, k, ts(m, 128)],
                                                 xb[:, k, :],
                                                 start=(k == 0), stop=(k == DT - 1))
                            ev = PK.tile([128, 512], F32, tag="kev", bufs=2)
                            nc.vector.tensor_tensor(ev[:], ps[:], r_bc[:], ALU.mult)
                            nc.vector.tensor_scalar(kT[:, m, off:off + 512], ev[:],
                                                    bk_t[:, m:m + 1], None, ALU.add)

                # ---------------- Phase V ----------------
                with tc.tile_pool(name="phv", bufs=2) as PV, \
                     tc.tile_pool(name="phv_ps", bufs=2, space="PSUM") as PVP:
                    wv_t = PV.tile([128, DT, D], F32R, bufs=1)
                    nc.sync.dma_start(wv_t[:], dram3(wv))
                    for blk in range(NBLK):
                        off = blk * 512
                        xb = PV.tile([128, DT, 512], F32R, tag="xb", bufs=2)
                        nc.sync.dma_start(xb[:], dram3(xT)[:, :, off:off + 512])
                        mu_bc = PV.tile([128, 512], F32, tag="mu_bc", bufs=2)
                        nc.gpsimd.partition_broadcast(mu_bc[:],
                                                      mu_row[0:1, off:off + 512])
                        for k in range(DT):
                            nc.vector.tensor_tensor(xb[:, k, :], xb[:, k, :],
                                                    mu_bc[:], ALU.subtract)
                        for tt in range(4):
                            g = blk * 4 + tt
                            for n in range(c.NVB):
                                ps = PVP.tile([128, c.VN], F32, tag="vps")
                                for k in range(DT):
                                    nc.tensor.matmul(
                                        ps[:],
                                        xb[:, k, ts(tt, 128)],
                                        wv_t[:, k, ts(n, c.VN)],
                                        start=(k == 0), stop=(k == DT - 1))
                                ev = PV.tile([128, c.VN], F32, tag="vev", bufs=2)
                                nc.vector.scalar_tensor_tensor(
                                    ev[:], ps[:], rt[:, g:g + 1],
                                    bvb[:, ts(n, c.VN)], ALU.mult, ALU.add)
                                dst = vone[:, g, n * c.HPV:(n + 1) * c.HPV, 0:64]
                                nc.vector.tensor_copy(
                                    dst, ev[:].rearrange("p (h c) -> p h c", c=64))

                # ---------------- Phase attention ----------------
                with tc.tile_pool(name="pha", bufs=2) as PA, \
                     tc.tile_pool(name="pha_ps", bufs=2, space="PSUM") as PAP:
                    mk = PA.tile([128, NTT, TQ], BF16, bufs=1)
                    nc.sync.dma_start(mk[:], maskT.ap().rearrange(
                        "(a p) q -> p a q", p=128))
                    for hp in range(c.NH // 2):
                        av0 = PAP.tile([65, TQ], F32, tag="av0", bufs=2)
                        av1 = PAP.tile([65, TQ], F32, tag="av1", bufs=2)
                        for tk in range(NTT):
                            s0 = PAP.tile([128, TQ], F32, tag="s0", bufs=2)
                            s1 = PAP.tile([128, TQ], F32, tag="s1", bufs=2)
                            nc.tensor.matmul(s0[:], kT[0:64, hp, ts(tk, 128)],
                                             qT[0:64, hp, :], start=True, stop=True)
                            nc.tensor.matmul(s1[:], kT[64:128, hp, ts(tk, 128)],
                                             qT[64:128, hp, :], start=True, stop=True)
                            e0 = PA.tile([128, TQ], BF16, tag="e0", bufs=3)
                            e1 = PA.tile([128, TQ], BF16, tag="e1", bufs=3)
                            nc.vector.tensor_tensor(e0[:], s0[:], mk[:, tk, :], ALU.add)
                            nc.vector.tensor_tensor(e1[:], s1[:], mk[:, tk, :], ALU.add)
                            p0 = PA.tile([128, TQ], BF16, tag="p0", bufs=3)
                            p1 = PA.tile([128, TQ], BF16, tag="p1", bufs=3)
                            nc.scalar.activation(p0[:], e0[:], AF.Exp)
                            nc.scalar.activation(p1[:], e1[:], AF.Exp)
                            h0, h1 = 2 * hp, 2 * hp + 1
                            nc.tensor.matmul(av0[:], V[:, tk, h0 * 65:(h0 + 1) * 65],
                                             p0[:], start=(tk == 0),
                                             stop=(tk == NTT - 1))
                            nc.tensor.matmul(av1[:], V[:, tk, h1 * 65:(h1 + 1) * 65],
                                             p1[:], start=(tk == 0),
                                             stop=(tk == NTT - 1))
                        for av, half in ((av0, 0), (av1, 1)):
                            rec = PA.tile([1, TQ], F32, tag="rec", bufs=2)
                            nc.vector.reciprocal(rec[:], av[64:65, :])
                            rbc = PA.tile([64, TQ], F32, tag="rbc", bufs=2)
                            nc.gpsimd.partition_broadcast(rbc[:], rec[:], channels=64)
                            nc.vector.tensor_tensor(
                                aoT[64 * half:64 * (half + 1), hp, :],
                                av[0:64, :], rbc[:], ALU.mult)

            # ---------------- Phase proj (+ residual) ----------------
            with tc.tile_pool(name="php", bufs=2) as PP, \
                 tc.tile_pool(name="php_ps", bufs=2, space="PSUM") as PPP:
                pw_t = PP.tile([128, DT, D], F32R, bufs=1)
                nc.sync.dma_start(pw_t[:], dram3(pw))
                xq2 = PP.tile([128, DT, TQ], F32R, bufs=1)
                nc.sync.dma_start(xq2[:], dram3(xqT))
                for m in range(DT):
                    ps = PPP.tile([128, TQ], F32, tag="pps")
                    for k in range(DT):
                        nc.tensor.matmul(ps[:], pw_t[:, k, ts(m, 128)],
                                         aoT[:, k, :],
                                         start=(k == 0), stop=(k == DT - 1))
                    nc.vector.scalar_tensor_tensor(x2T[:, m, :], ps[:],
                                                   pb_t[:, m:m + 1], xq2[:, m, :],
                                                   ALU.add, ALU.add)

            # ---------------- Phase FFN ----------------
            with tc.tile_pool(name="phf", bufs=2) as PF, \
                 tc.tile_pool(name="phf_ps", bufs=2, space="PSUM") as PFP:
                h = PF.tile([128, FT, TQ], F32R, bufs=1)
                mu2_r = PF.tile([1, TQ], F32, bufs=1)
                r2_r = PF.tile([1, TQ], F32, bufs=1)
                mu2_bc = PF.tile([128, TQ], F32, bufs=1)
                r2_bc = PF.tile([128, TQ], F32, bufs=1)
                stats_center(x2T, TQ, mu2_r[:], r2_r[:], mu2_bc, r2_bc, PF, PFP)
                for m in range(FT):
                    w1c = PF.tile([128, DT, 128], F32R, tag="w1c", bufs=4)
                    nc.sync.dma_start(w1c[:], dram3(w1)[:, :, ts(m, 128)])
                    ps = PFP.tile([128, TQ], F32, tag="hps")
                    for k in range(DT):
                        nc.tensor.matmul(ps[:], w1c[:, k, :], x2T[:, k, :],
                                         start=(k == 0), stop=(k == DT - 1))
                    th = PF.tile([128, TQ], F32, tag="th", bufs=2)
                    nc.vector.tensor_tensor(th[:], ps[:], r2_bc[:], ALU.mult)
                    nc.scalar.activation(h[:, m, :], th[:], AF.Gelu,
                                         bias=b1_t[:, m:m + 1])
                for m in range(DT):
                    w2c = PF.tile([128, FT, 128], F32R, tag="w2c", bufs=2)
                    nc.sync.dma_start(w2c[:], dram3(w2)[:, :, ts(m, 128)])
                    ps = PFP.tile([128, TQ], F32, tag="ops")
                    for k in range(FT):
                        nc.tensor.matmul(ps[:], w2c[:, k, :], h[:, k, :],
                                         start=(k == 0), stop=(k == FT - 1))
                    t2 = PF.tile([128, TQ], F32, tag="t2", bufs=2)
                    nc.vector.scalar_tensor_tensor(t2[:], ps[:], b2_t[:, m:m + 1],
                                                   x2T[:, m, :], ALU.add, ALU.add)
                    ob = PF.tile([128, TQ], F32, tag="ob", bufs=2)
                    nc.vector.tensor_tensor(ob[:], t2[:], mu2_bc[:], ALU.add)
                    nc.sync.dma_start(
                        outT.ap().rearrange("(a p) t -> p a t", p=128)[:, m, :],
                        ob[:])
    nc.compile()
    return nc


_CACHE = {}


def _get_nc(c: CFG):
    key = (c.D, c.TB, c.TQ, c.NH, c.HFF)
    if key not in _CACHE:
        _CACHE[key] = build_nc(c)
    return _CACHE[key]


def make_in_maps(c: CFG, x, mask, ln1_g, ln1_b, qkv_w, qkv_b, proj_w, proj_b,
                 ln2_g, ln2_b, w1, b1, w2, b2):
    D, TB, TQ, DT, FT = c.D, c.TB, c.TQ, c.DT, c.FT
    B = x.shape[0]
    ncg = TB // TQ  # query groups per batch

    f = np.float32
    bf = ml_dtypes.bfloat16
    g1 = ln1_g.astype(f)
    sc = 1.0 / np.sqrt(c.HD)
    wq_f = np.ascontiguousarray(qkv_w[:, :D] * g1[:, None] * sc, f)
    wk_f = np.ascontiguousarray(qkv_w[:, D:2 * D] * g1[:, None], f)
    wv_f = np.ascontiguousarray(qkv_w[:, 2 * D:] * g1[:, None], f)
    bq_f = ((qkv_b[:D] + ln1_b @ qkv_w[:, :D]) * sc).astype(f)
    bk_f = (qkv_b[D:2 * D] + ln1_b @ qkv_w[:, D:2 * D]).astype(f)
    bv_f = (qkv_b[2 * D:] + ln1_b @ qkv_w[:, 2 * D:]).astype(f)
    w1_f = np.ascontiguousarray(w1 * ln2_g.astype(f)[:, None], f)
    b1_f = (b1 + ln2_b @ w1).astype(f)
    pw_f = np.ascontiguousarray(proj_w, f)
    w2_f = np.ascontiguousarray(w2, f)

    def btile(v, nt):
        return np.ascontiguousarray(v.reshape(nt, 128).T, f)

    madd = ((mask[0, 0].astype(np.float32) - 1.0) * 1e9).astype(np.float32)  # [T,T]

    shared = {
        "wq": wq_f, "wk": wk_f, "wv": wv_f, "pw": pw_f,
        "w1": w1_f, "w2": w2_f,
        "bq": btile(bq_f, DT), "bk": btile(bk_f, DT),
        "bv": np.ascontiguousarray(bv_f.reshape(1, D)),
        "pb": btile(proj_b.astype(f), DT),
        "b1": btile(b1_f, FT), "b2": btile(b2.astype(f), DT),
    }
    in_maps = []
    for core in range(B * ncg):
        b, j = core // ncg, core % ncg
        qs = j * TQ
        xTb = np.ascontiguousarray(x[b].T, f)                     # [D, TB]
        m = dict(shared)
        m["xT"] = xTb
        m["xqT"] = np.ascontiguousarray(x[b, qs:qs + TQ, :].T, f)  # [D, TQ]
        m["maskT"] = np.ascontiguousarray(madd[qs:qs + TQ, :].T.astype(bf))
        in_maps.append(m)
    return in_maps


def assemble_out(c: CFG, results, B):
    ncg = c.TB // c.TQ
    out = np.empty((B, c.TB, c.D), np.float32)
    for core, res in enumerate(results):
        b, j = core // ncg, core % ncg
        out[b, j * c.TQ:(j + 1) * c.TQ, :] = res["outT"].T
    return out


def kernel(x, mask, ln1_g, ln1_b, qkv_w, qkv_b, proj_w, proj_b,
           ln2_g, ln2_b, w1, b1, w2, b2):
    x = np.asarray(x, np.float32)
    c = CFG(D=x.shape[2], TB=x.shape[1], TQ=x.shape[1] // 4,
            NH=16, HD=64, HFF=4 * x.shape[2])
    nc = _get_nc(c)
    in_maps = make_in_maps(c, x, np.asarray(mask), *[np.asarray(a, np.float32)
                           for a in (ln1_g, ln1_b, qkv_w, qkv_b, proj_w, proj_b,
                                     ln2_g, ln2_b, w1, b1, w2, b2)])
    res = run_bass_kernel_spmd(nc, in_maps, core_ids=list(range(len(in_maps))))
    return assemble_out(c, res.results, x.shape[0])


if __name__ == "__main__":
    c = CFG()
    nc = build_nc(c)
    print("built ok")
